# revision 10
# baseline (speedup 1.0000x reference)
"""Trainium2 Bass kernel for a 2-layer masked LSTM + FC + sigmoid head.

Problem shapes (hardcoded): B=1024, T=512, I=16, H=64.
Sharding: pure data parallel, batch 1024 -> 8 cores x 128.

Per-core design (v4 — bf16 matmul path, single sigmoid, Pool fc)
----------------------------------------------------------------
All matmul operands are bf16 (1 PE cycle/row at any moving size, vs
fp32r's >=256 requirement), so the zero-half trick is gone: per step
each layer runs two F=128 matmuls into disjoint PSUM column ranges
[IF | OG] of a [128, 512] bank (2 steps per bank, 2 banks per layer).

Layer 0 needs no separate input projection: gates = [W_hh0; b0; W_ih0]
@ [h0(t-1); 1; x(t)] as a K=81 matmul per gate-block. The rhs is an
8-slot ring [81, 1024] bf16 (slot = 128 cols; h0 rows 0:64, ones row
64, x rows 65:81). Layer 1 lags by 4 steps; its input projection
W_ih1 @ h0 + b1 is batched per step-pair into the IF1/OG1 columns
(start=True), the K=64 recurrent matmuls accumulate on top.

One sigmoid ACT [128, 256] per layer per step covers i, f, 2g, o
(tanh(g) = 2*sigmoid(2g) - 1 with the 2x folded into the g weights);
ig2 needs both halves anyway so splitting only serializes the ACT
queue. Cell update per layer: fc = f*c on GpSimd (parallel with the
Vector engine's ig2 = (sig2g-.5)*i), then c' = 2*ig2 + fc, tanh, and
the h-write (f32 -> bf16 ring cast) on Vector. Emission order per
step keeps h0w ahead of layer-1's trio in the Vector queue.

Masked final-state capture: run unmasked; since d[b,t] = mask[b,t] -
mask[b,t+1] is one-hot over t, capture is a single copy_predicated
per 4-step block (last-write-wins, no accumulate): h2cap[64, 512]
gets h1(t) at each sequence's own length.
"""

from contextlib import ExitStack

import numpy as np
import ml_dtypes

import concourse.bass as bass
import concourse.tile as tile
from concourse import bacc, mybir
from concourse import bass_utils

F32 = mybir.dt.float32
BF16 = mybir.dt.bfloat16
F32R = mybir.dt.float32r
AF = mybir.ActivationFunctionType
OP = mybir.AluOpType

B, T, I, H = 1024, 512, 16, 64
NCORES = 8
BL = B // NCORES  # 128 batch per core
LAG = 4           # layer-1 step lag

_BUILT = {}


def _build_program(t_steps: int):
    nc = bacc.Bacc(
        "TRN2",
        target_bir_lowering=False,
        debug=False,
        enable_asserts=False,
        num_devices=NCORES,
    )

    TB = t_steps * BL  # 65536
    d_xs = nc.dram_tensor("xs", [16, TB], BF16, kind="ExternalInput")
    d_dbc = nc.dram_tensor("dbc", [64, TB], mybir.dt.uint8, kind="ExternalInput")
    wnames = [
        ("wif0", 81), ("wog0", 81),      # [W_hh0; b0; W_ih0] fused lhsT
        ("w0if1", 65), ("w0og1", 65),    # [W_ih1; b1] rider lhsT
        ("whif1", 64), ("whog1", 64),    # W_hh1 lhsT
    ]
    d_w = {}
    for name, k in wnames:
        d_w[name] = nc.dram_tensor(name, [k, 128], BF16, kind="ExternalInput")
    d_fct = nc.dram_tensor("fct", [64, 1], BF16, kind="ExternalInput")
    d_fcb = nc.dram_tensor("fcb", [1, 1], F32, kind="ExternalInput")
    d_out = nc.dram_tensor("out", [1, 128], F32, kind="ExternalOutput")

    NCH = TB // 2048  # 32 dbc chunks of 16 steps

    with tile.TileContext(nc) as tc, ExitStack() as ctx:
        pconst = ctx.enter_context(tc.tile_pool(name="const", bufs=1))
        pstate = ctx.enter_context(tc.tile_pool(name="state", bufs=1))
        ppsum = ctx.enter_context(tc.tile_pool(name="psum", bufs=1, space="PSUM"))
        pwork = ctx.enter_context(tc.tile_pool(name="work", bufs=3))

        # ---- weights ----
        w = {}
        for name, k in wnames:
            w[name] = pconst.tile([k, 128], BF16, tag=name, name=name)
            nc.sync.dma_start(w[name][:], d_w[name].ap()[:])
        fct = pconst.tile([64, 1], BF16, tag="fct")
        nc.sync.dma_start(fct[:], d_fct.ap()[:])
        fcb = pconst.tile([1, 1], F32, tag="fcb")
        nc.sync.dma_start(fcb[:], d_fcb.ap()[:])

        # ---- rings: slot s = 128 cols ----
        # ring0: rows 0:64 h0(s), row 64 ones, rows 65:81 x(s+1)
        ring0 = pstate.tile([81, 8 * 128], BF16, tag="ring0")
        nc.vector.memset(ring0[:], 0.0)
        nc.vector.memset(ring0[64:65, :], 1.0)
        # ring1: rows 0:64 h1(s)
        ring1 = pstate.tile([64, 8 * 128], BF16, tag="ring1")
        nc.vector.memset(ring1[:], 0.0)

        dcb = [pconst.tile([64, 2048], mybir.dt.uint8, tag=f"dcb{i}", name=f"dcb{i}")
               for i in range(3)]
        for j in range(2):
            nc.sync.dma_start(dcb[j][:], d_dbc.ap()[:, j * 2048:(j + 1) * 2048])

        def dma_x(t0, n):
            # x(t0..t0+n-1) -> ring0 slots (t0-1)%8 .. contiguous, rows 65:81
            s0 = (t0 - 1) % 8
            dst = ring0[65:81, 128 * s0:128 * (s0 + n)]
            nc.sync.dma_start(dst, d_xs.ap()[:, t0 * BL:(t0 + n) * BL])

        dma_x(0, 1)   # x(0) -> slot 7
        dma_x(1, 4)   # slots 0..3
        dma_x(5, 4)   # slots 4..7

        c_sb = pstate.tile([128, 256], BF16, tag="csb")
        nc.vector.memset(c_sb[:], 0.0)
        # sig1's bias operand is rewritten (to 0) after each step's c'0:
        # this delays sig1 past c'0 in BOTH the tile scheduler's cost model
        # and on hardware, so tanh0 wins the ACT queue and h0w wins the
        # Vector queue ahead of layer-1's trio (which otherwise wedges in
        # front of them and stretches the critical chain).
        zconst = pconst.tile([128, 1], BF16, tag="zconst")
        nc.vector.memset(zconst[:], 0.0)
        biasg = pstate.tile([128, 1], BF16, tag="biasg")
        nc.vector.memset(biasg[:], 0.0)
        h2cap = pstate.tile([64, 512], BF16, tag="h2cap")
        nc.vector.memset(h2cap[:], 0.0)

        # ---- PSUM: per-layer step-pair banks ----
        pg0 = [ppsum.tile([128, 512], F32, tag=f"pg0{i}", name=f"pg0{i}")
               for i in range(2)]
        pg1 = [ppsum.tile([128, 512], F32, tag=f"pg1{i}", name=f"pg1{i}")
               for i in range(2)]

        mm = nc.tensor.matmul

        for k in range(t_steps + LAG):
            t = k
            tau = k - LAG
            l0 = t < t_steps
            l1 = 0 <= tau < t_steps
            bk0 = pg0[(k // 2) % 2]
            s0c = 256 * (k % 2)
            bk1 = pg1[((k - LAG) // 2) % 2] if k >= LAG else None
            s1c = 256 * ((k - LAG) % 2) if k >= LAG else 0

            # ---- layer-0 matmuls (chain leader) ----
            if l0:
                sp = (t - 1) % 8
                rhs = ring0[0:81, 128 * sp:128 * (sp + 1)]
                mm(bk0[:, s0c:s0c + 128], w["wif0"][:], rhs, start=True,
                   stop=True, skip_group_check=True)
                mm(bk0[:, s0c + 128:s0c + 256], w["wog0"][:], rhs, start=True,
                   stop=True, skip_group_check=True)

            # ---- layer-1 recurrent matmuls ----
            if l1:
                sp = (tau - 1) % 8
                rhs = ring1[0:64, 128 * sp:128 * (sp + 1)]
                mm(bk1[:, s1c:s1c + 128], w["whif1"][:], rhs, start=False,
                   stop=True, skip_group_check=True)
                mm(bk1[:, s1c + 128:s1c + 256], w["whog1"][:], rhs, start=False,
                   stop=True, skip_group_check=True)

            # ---- sigmoids (one per layer) ----
            g_sb = pwork.tile([128, 512], BF16, tag="gsb")
            tc_sb = pwork.tile([128, 256], BF16, tag="tcsb")
            fco = pwork.tile([64, 256], BF16, tag="fco")
            ig2 = pwork.tile([64, 256], BF16, tag="ig2")
            if l0:
                nc.scalar.activation(g_sb[:, 0:256], bk0[:, s0c:s0c + 256],
                                     AF.Sigmoid)


            # cell state is c~ = c/2 (the 2x is folded into tanh's scale),
            # so c' is a plain TT add: c~ = f*c~ + (sig2g - .5)*i
            def cell(ell, ring, st):
                b0c = 256 * ell
                co = 128 * ell
                cin = c_sb[64:128, co:co + 128]
                nc.vector.tensor_tensor(fco[:, co:co + 128],
                                        g_sb[64:128, b0c:b0c + 128], cin,
                                        OP.mult)
                nc.vector.scalar_tensor_tensor(ig2[:, co:co + 128],
                                               g_sb[0:64, b0c + 128:b0c + 256], 0.5,
                                               g_sb[0:64, b0c:b0c + 128],
                                               OP.subtract, OP.mult)
                nc.vector.tensor_tensor(cin, ig2[:, co:co + 128],
                                        fco[:, co:co + 128], OP.add)
                if ell == 0 and l1:
                    # rewrite sig1's bias (to 0.0) with a read of sig0's
                    # output column, on the idle GpSimd engine: sig1 then
                    # fires in the ACT idle gap between sig0 and tanh0,
                    # and layer-1's trio drains before h0w needs the DVE
                    nc.gpsimd.tensor_tensor(biasg[:], g_sb[0:128, 0:1],
                                            zconst[:], OP.mult)
                nc.scalar.activation(tc_sb[64:128, co:co + 128], cin, AF.Tanh,
                                     scale=2.0)
                dst = ring[0:64, 128 * st:128 * (st + 1)]
                nc.vector.tensor_tensor(dst, g_sb[64:128, b0c + 128:b0c + 256],
                                        tc_sb[64:128, co:co + 128], OP.mult)

            if l0:
                cell(0, ring0, t % 8)
            if l1:
                # emitted after cell(0) so the bias operand binds to THIS
                # step's biasw (written just after c'0)
                nc.scalar.activation(g_sb[:, 256:512], bk1[:, s1c:s1c + 256],
                                     AF.Sigmoid,
                                     bias=biasg[:] if l0 else 0.0)
                cell(1, ring1, tau % 8)

            # ---- capture: one-hot mask -> predicated copy, per 4 steps ----
            if k % 4 == 3 and k >= LAG + 3:
                c = (k - 3 - LAG) // 4
                t0 = 4 * c
                ch = dcb[(t0 // 16) % 3]
                dsl = ch[:, (t0 % 16) * 128:(t0 % 16) * 128 + 512]
                sp = t0 % 8
                h1s = ring1[0:64, 128 * sp:128 * (sp + 4)]
                nc.vector.copy_predicated(h2cap[:, 0:256], dsl[:, 0:256],
                                          h1s[:, 0:256])
                nc.vector.copy_predicated(h2cap[:, 256:512], dsl[:, 256:512],
                                          h1s[:, 256:512])

            # ---- layer-1 input projection, 3 steps ahead of use ----
            if k % 2 == 1:
                q = (k - 1) // 2
                if 0 <= q < t_steps // 2:
                    tb = pg1[q % 2]
                    rr = tb[:].rearrange("p (s c) -> p s c", c=256)
                    t0 = 2 * q
                    sp = t0 % 8
                    rhs = ring0[0:65, 128 * sp:128 * (sp + 2)]
                    mm(rr[:, :, 0:128], w["w0if1"][:], rhs, start=True,
                       stop=False, skip_group_check=True)
                    mm(rr[:, :, 128:256], w["w0og1"][:], rhs, start=True,
                       stop=False, skip_group_check=True)

            # ---- input streaming ----
            if k % 4 == 0 and k >= 8 and k + 1 < t_steps:
                n = min(4, t_steps - (k + 1))
                dma_x(k + 1, n)
            if k % 16 == 4 and k // 16 + 2 < NCH:
                j = k // 16 + 2
                nc.sync.dma_start(dcb[j % 3][:], d_dbc.ap()[:, j * 2048:(j + 1) * 2048])

        # ---------- FC + sigmoid head ----------
        hfold = pwork.tile([64, 256], BF16, tag="hfold")
        nc.vector.tensor_tensor(hfold[:], h2cap[:, 0:256], h2cap[:, 256:512], OP.add)
        h2 = pwork.tile([64, 128], BF16, tag="h2")
        nc.vector.tensor_tensor(h2[:], hfold[:, 0:128], hfold[:, 128:256], OP.add)
        mm(pg0[0][0:1, 0:128], fct[:], h2[:], start=True, stop=True,
           skip_group_check=True)
        osb = pwork.tile([1, 128], F32, tag="osb")
        nc.scalar.activation(osb[:], pg0[0][0:1, 0:128], AF.Sigmoid, bias=fcb[:, 0:1])
        nc.sync.dma_start(d_out.ap()[:], osb[:])

    nc.compile()
    return nc


def _get_program(t_steps: int):
    if t_steps not in _BUILT:
        _BUILT[t_steps] = _build_program(t_steps)
    return _BUILT[t_steps]


def _prep_core_inputs(x, dmask, weights, t_steps):
    """Host-side layout prep for one core's shard. x: [BL, T, I], dmask: [BL, T]."""
    TB = t_steps * BL
    xs = np.ascontiguousarray(
        np.asarray(x, np.float32).transpose(2, 1, 0).reshape(16, TB)
    ).astype(ml_dtypes.bfloat16)
    dbc = np.ascontiguousarray(
        np.broadcast_to(dmask.T.reshape(1, TB), (64, TB))
    ).astype(np.uint8)
    return dict(xs=xs, dbc=dbc, **weights)


def _host_weights(w_ih0, w_hh0, b_ih0, b_hh0,
                  w_ih1, w_hh1, b_ih1, b_hh1, fc_w, fc_b):
    b0 = np.asarray(b_ih0, np.float32) + np.asarray(b_hh0, np.float32)
    b1 = np.asarray(b_ih1, np.float32) + np.asarray(b_hh1, np.float32)
    wih0, whh0 = np.asarray(w_ih0, np.float32), np.asarray(w_hh0, np.float32)
    wih1, whh1 = np.asarray(w_ih1, np.float32), np.asarray(w_hh1, np.float32)

    def og_w(wm):  # [4H, K] -> [2g; o] stacked [128, K] (PyTorch i,f,g,o rows)
        return np.concatenate([2.0 * wm[2 * H:3 * H], wm[3 * H:4 * H]], axis=0)

    def og_b(bv):
        return np.concatenate([2.0 * bv[2 * H:3 * H], bv[3 * H:4 * H]])

    def fused0(wx, wh, bv):  # [W_hh; b; W_ih] lhsT [81, 128]
        out = np.empty((81, 128), np.float32)
        out[0:64] = wh.T
        out[64] = bv
        out[65:81] = wx.T
        return out

    def rider1(wx, bv):  # [W_ih1; b1] lhsT [65, 128]
        out = np.empty((65, 128), np.float32)
        out[0:64] = wx.T
        out[64] = bv
        return out

    bf = ml_dtypes.bfloat16
    weights = dict(
        wif0=fused0(wih0[0:2 * H], whh0[0:2 * H], b0[0:2 * H]).astype(bf),
        wog0=fused0(og_w(wih0), og_w(whh0), og_b(b0)).astype(bf),
        w0if1=rider1(wih1[0:2 * H], b1[0:2 * H]).astype(bf),
        w0og1=rider1(og_w(wih1), og_b(b1)).astype(bf),
        whif1=np.ascontiguousarray(whh1[0:2 * H].T).astype(bf),
        whog1=np.ascontiguousarray(og_w(whh1).T).astype(bf),
        fct=np.ascontiguousarray(
            np.asarray(fc_w, np.float32).reshape(1, H).T).astype(bf),
        fcb=np.asarray(fc_b, np.float32).reshape(1, 1),
    )
    return weights


def _run(x, mask, w_ih0, w_hh0, b_ih0, b_hh0,
         w_ih1, w_hh1, b_ih1, b_hh1, fc_w, fc_b, trace=False):
    t_steps = x.shape[1]
    x = np.asarray(x, np.float32)
    mask = np.asarray(mask)

    # d[b, t] = mask[b, t] - mask[b, t+1]  (one-hot at t = len_b - 1)
    m = mask.astype(np.float32)
    d = m - np.concatenate([m[:, 1:], np.zeros((m.shape[0], 1), np.float32)], axis=1)

    weights = _host_weights(w_ih0, w_hh0, b_ih0, b_hh0,
                            w_ih1, w_hh1, b_ih1, b_hh1, fc_w, fc_b)

    nc = _get_program(t_steps)
    in_maps = []
    for c in range(NCORES):
        sl = slice(c * BL, (c + 1) * BL)
        in_maps.append(_prep_core_inputs(x[sl], d[sl], weights, t_steps))

    res = bass_utils.run_bass_kernel_spmd(nc, in_maps, core_ids=list(range(NCORES)),
                                          trace=trace)
    out = np.concatenate([res.results[c]["out"].reshape(BL) for c in range(NCORES)])
    return out.astype(np.float32), res


def kernel(**inputs):
    return _run(**inputs)[0]


def kernel_traced(**inputs):
    return _run(**inputs, trace=True)


# revision 11
# speedup vs baseline: 1.0052x; 1.0052x over previous
"""Trainium2 Bass kernel for a 2-layer masked LSTM + FC + sigmoid head.

Problem shapes (hardcoded): B=1024, T=512, I=16, H=64.
Sharding: pure data parallel, batch 1024 -> 8 cores x 128.

Per-core design (v4 — bf16 matmul path, single sigmoid, Pool fc)
----------------------------------------------------------------
All matmul operands are bf16 (1 PE cycle/row at any moving size, vs
fp32r's >=256 requirement), so the zero-half trick is gone: per step
each layer runs two F=128 matmuls into disjoint PSUM column ranges
[IF | OG] of a [128, 512] bank (2 steps per bank, 2 banks per layer).

Layer 0 needs no separate input projection: gates = [W_hh0; b0; W_ih0]
@ [h0(t-1); 1; x(t)] as a K=81 matmul per gate-block. The rhs is an
8-slot ring [81, 1024] bf16 (slot = 128 cols; h0 rows 0:64, ones row
64, x rows 65:81). Layer 1 lags by 4 steps; its input projection
W_ih1 @ h0 + b1 is batched per step-pair into the IF1/OG1 columns
(start=True), the K=64 recurrent matmuls accumulate on top.

One sigmoid ACT [128, 256] per layer per step covers i, f, 2g, o
(tanh(g) = 2*sigmoid(2g) - 1 with the 2x folded into the g weights);
ig2 needs both halves anyway so splitting only serializes the ACT
queue. Cell update per layer: fc = f*c on GpSimd (parallel with the
Vector engine's ig2 = (sig2g-.5)*i), then c' = 2*ig2 + fc, tanh, and
the h-write (f32 -> bf16 ring cast) on Vector. Emission order per
step keeps h0w ahead of layer-1's trio in the Vector queue.

Masked final-state capture: run unmasked; since d[b,t] = mask[b,t] -
mask[b,t+1] is one-hot over t, capture is a single copy_predicated
per 4-step block (last-write-wins, no accumulate): h2cap[64, 512]
gets h1(t) at each sequence's own length.
"""

from contextlib import ExitStack

import numpy as np
import ml_dtypes

import concourse.bass as bass
import concourse.tile as tile
from concourse import bacc, mybir
from concourse import bass_utils

F32 = mybir.dt.float32
BF16 = mybir.dt.bfloat16
F32R = mybir.dt.float32r
AF = mybir.ActivationFunctionType
OP = mybir.AluOpType

B, T, I, H = 1024, 512, 16, 64
NCORES = 8
BL = B // NCORES  # 128 batch per core
LAG = 4           # layer-1 step lag

_BUILT = {}


def _build_program(t_steps: int):
    nc = bacc.Bacc(
        "TRN2",
        target_bir_lowering=False,
        debug=False,
        enable_asserts=False,
        num_devices=NCORES,
    )

    TB = t_steps * BL  # 65536
    d_xs = nc.dram_tensor("xs", [16, TB], BF16, kind="ExternalInput")
    d_dbc = nc.dram_tensor("dbc", [64, TB], BF16, kind="ExternalInput")
    wnames = [
        ("wif0", 81), ("wog0", 81),      # [W_hh0; b0; W_ih0] fused lhsT
        ("w0if1", 65), ("w0og1", 65),    # [W_ih1; b1] rider lhsT
        ("whif1", 64), ("whog1", 64),    # W_hh1 lhsT
    ]
    d_w = {}
    for name, k in wnames:
        d_w[name] = nc.dram_tensor(name, [k, 128], BF16, kind="ExternalInput")
    d_fct = nc.dram_tensor("fct", [64, 1], BF16, kind="ExternalInput")
    d_fcb = nc.dram_tensor("fcb", [1, 1], F32, kind="ExternalInput")
    d_out = nc.dram_tensor("out", [1, 128], F32, kind="ExternalOutput")

    NCH = TB // 2048  # 32 dbc chunks of 16 steps

    with tile.TileContext(nc) as tc, ExitStack() as ctx:
        pconst = ctx.enter_context(tc.tile_pool(name="const", bufs=1))
        pstate = ctx.enter_context(tc.tile_pool(name="state", bufs=1))
        ppsum = ctx.enter_context(tc.tile_pool(name="psum", bufs=1, space="PSUM"))
        pwork = ctx.enter_context(tc.tile_pool(name="work", bufs=3))

        # ---- weights ----
        w = {}
        for name, k in wnames:
            w[name] = pconst.tile([k, 128], BF16, tag=name, name=name)
            nc.sync.dma_start(w[name][:], d_w[name].ap()[:])
        fct = pconst.tile([64, 1], BF16, tag="fct")
        nc.sync.dma_start(fct[:], d_fct.ap()[:])
        fcb = pconst.tile([1, 1], F32, tag="fcb")
        nc.sync.dma_start(fcb[:], d_fcb.ap()[:])

        # ---- rings: slot s = 128 cols ----
        # ring0: rows 0:64 h0(s), row 64 ones, rows 65:81 x(s+1)
        ring0 = pstate.tile([81, 8 * 128], BF16, tag="ring0")
        nc.vector.memset(ring0[:], 0.0)
        nc.vector.memset(ring0[64:65, :], 1.0)
        # ring1: rows 0:64 h1(s)
        ring1 = pstate.tile([64, 8 * 128], BF16, tag="ring1")
        nc.vector.memset(ring1[:], 0.0)

        dcb = [pconst.tile([64, 2048], BF16, tag=f"dcb{i}", name=f"dcb{i}")
               for i in range(3)]
        for j in range(2):
            nc.sync.dma_start(dcb[j][:], d_dbc.ap()[:, j * 2048:(j + 1) * 2048])

        def dma_x(t0, n):
            # x(t0..t0+n-1) -> ring0 slots (t0-1)%8 .. contiguous, rows 65:81
            s0 = (t0 - 1) % 8
            dst = ring0[65:81, 128 * s0:128 * (s0 + n)]
            nc.sync.dma_start(dst, d_xs.ap()[:, t0 * BL:(t0 + n) * BL])

        dma_x(0, 1)   # x(0) -> slot 7
        dma_x(1, 4)   # slots 0..3
        dma_x(5, 4)   # slots 4..7

        c_sb = pstate.tile([128, 256], BF16, tag="csb")
        nc.vector.memset(c_sb[:], 0.0)
        # sig1's bias operand is rewritten (to 0) after each step's c'0:
        # this delays sig1 past c'0 in BOTH the tile scheduler's cost model
        # and on hardware, so tanh0 wins the ACT queue and h0w wins the
        # Vector queue ahead of layer-1's trio (which otherwise wedges in
        # front of them and stretches the critical chain).
        zconst = pconst.tile([128, 1], BF16, tag="zconst")
        nc.vector.memset(zconst[:], 0.0)
        biasg = pstate.tile([128, 1], BF16, tag="biasg")
        nc.vector.memset(biasg[:], 0.0)
        h2cap = pstate.tile([64, 512], BF16, tag="h2cap")
        nc.vector.memset(h2cap[:], 0.0)

        # ---- PSUM: per-layer step-pair banks ----
        pg0 = [ppsum.tile([128, 512], F32, tag=f"pg0{i}", name=f"pg0{i}")
               for i in range(2)]
        pg1 = [ppsum.tile([128, 512], F32, tag=f"pg1{i}", name=f"pg1{i}")
               for i in range(2)]

        mm = nc.tensor.matmul

        for k in range(t_steps + LAG):
            t = k
            tau = k - LAG
            l0 = t < t_steps
            l1 = 0 <= tau < t_steps
            bk0 = pg0[(k // 2) % 2]
            s0c = 256 * (k % 2)
            bk1 = pg1[((k - LAG) // 2) % 2] if k >= LAG else None
            s1c = 256 * ((k - LAG) % 2) if k >= LAG else 0

            # ---- layer-0 matmuls (chain leader) ----
            if l0:
                sp = (t - 1) % 8
                rhs = ring0[0:81, 128 * sp:128 * (sp + 1)]
                mm(bk0[:, s0c:s0c + 128], w["wif0"][:], rhs, start=True,
                   stop=True, skip_group_check=True)
                mm(bk0[:, s0c + 128:s0c + 256], w["wog0"][:], rhs, start=True,
                   stop=True, skip_group_check=True)

            # ---- layer-1 recurrent matmuls ----
            if l1:
                sp = (tau - 1) % 8
                rhs = ring1[0:64, 128 * sp:128 * (sp + 1)]
                mm(bk1[:, s1c:s1c + 128], w["whif1"][:], rhs, start=False,
                   stop=True, skip_group_check=True)
                mm(bk1[:, s1c + 128:s1c + 256], w["whog1"][:], rhs, start=False,
                   stop=True, skip_group_check=True)

            # ---- sigmoids (one per layer) ----
            g_sb = pwork.tile([128, 512], BF16, tag="gsb")
            tc_sb = pwork.tile([128, 256], BF16, tag="tcsb")
            fco = pwork.tile([64, 256], BF16, tag="fco")
            ig2 = pwork.tile([64, 256], BF16, tag="ig2")
            if l0:
                nc.scalar.activation(g_sb[:, 0:256], bk0[:, s0c:s0c + 256],
                                     AF.Sigmoid)


            # cell state is c~ = c/2 (the 2x is folded into tanh's scale),
            # so c' is a plain TT add: c~ = f*c~ + (sig2g - .5)*i
            def cell(ell, ring, st):
                b0c = 256 * ell
                co = 128 * ell
                cin = c_sb[64:128, co:co + 128]
                nc.vector.tensor_tensor(fco[:, co:co + 128],
                                        g_sb[64:128, b0c:b0c + 128], cin,
                                        OP.mult)
                nc.vector.scalar_tensor_tensor(ig2[:, co:co + 128],
                                               g_sb[0:64, b0c + 128:b0c + 256], 0.5,
                                               g_sb[0:64, b0c:b0c + 128],
                                               OP.subtract, OP.mult)
                nc.vector.tensor_tensor(cin, ig2[:, co:co + 128],
                                        fco[:, co:co + 128], OP.add)
                if ell == 0 and l1:
                    # rewrite sig1's bias (to 0.0) with a read of c'0's
                    # output column: the next step's sig1 now trails c'0
                    nc.vector.scalar_tensor_tensor(biasg[:], c_sb[0:128, 0:1],
                                                   0.0, zconst[:],
                                                   OP.mult, OP.add)
                nc.scalar.activation(tc_sb[64:128, co:co + 128], cin, AF.Tanh,
                                     scale=2.0)
                dst = ring[0:64, 128 * st:128 * (st + 1)]
                nc.vector.tensor_tensor(dst, g_sb[64:128, b0c + 128:b0c + 256],
                                        tc_sb[64:128, co:co + 128], OP.mult)

            if l0:
                cell(0, ring0, t % 8)
            if l1:
                # emitted after cell(0) so the bias operand binds to THIS
                # step's biasw (written just after c'0)
                nc.scalar.activation(g_sb[:, 256:512], bk1[:, s1c:s1c + 256],
                                     AF.Sigmoid,
                                     bias=biasg[:] if l0 else 0.0)
                cell(1, ring1, tau % 8)

            # ---- capture: one-hot mask -> predicated copy, per 4 steps ----
            if k % 4 == 3 and k >= LAG + 3:
                c = (k - 3 - LAG) // 4
                t0 = 4 * c
                ch = dcb[(t0 // 16) % 3]
                dsl = ch[:, (t0 % 16) * 128:(t0 % 16) * 128 + 512]
                sp = t0 % 8
                h1s = ring1[0:64, 128 * sp:128 * (sp + 4)]
                mblk = pwork.tile([64, 512], BF16, tag="mblk")
                nc.gpsimd.tensor_tensor(mblk[:], dsl, h1s, OP.mult)
                nc.gpsimd.tensor_tensor(h2cap[:], h2cap[:], mblk[:], OP.add)

            # ---- layer-1 input projection, 3 steps ahead of use ----
            if k % 2 == 1:
                q = (k - 1) // 2
                if 0 <= q < t_steps // 2:
                    tb = pg1[q % 2]
                    rr = tb[:].rearrange("p (s c) -> p s c", c=256)
                    t0 = 2 * q
                    sp = t0 % 8
                    rhs = ring0[0:65, 128 * sp:128 * (sp + 2)]
                    mm(rr[:, :, 0:128], w["w0if1"][:], rhs, start=True,
                       stop=False, skip_group_check=True)
                    mm(rr[:, :, 128:256], w["w0og1"][:], rhs, start=True,
                       stop=False, skip_group_check=True)

            # ---- input streaming ----
            if k % 4 == 0 and k >= 8 and k + 1 < t_steps:
                n = min(4, t_steps - (k + 1))
                dma_x(k + 1, n)
            if k % 16 == 4 and k // 16 + 2 < NCH:
                j = k // 16 + 2
                nc.sync.dma_start(dcb[j % 3][:], d_dbc.ap()[:, j * 2048:(j + 1) * 2048])

        # ---------- FC + sigmoid head ----------
        hfold = pwork.tile([64, 256], BF16, tag="hfold")
        nc.vector.tensor_tensor(hfold[:], h2cap[:, 0:256], h2cap[:, 256:512], OP.add)
        h2 = pwork.tile([64, 128], BF16, tag="h2")
        nc.vector.tensor_tensor(h2[:], hfold[:, 0:128], hfold[:, 128:256], OP.add)
        mm(pg0[0][0:1, 0:128], fct[:], h2[:], start=True, stop=True,
           skip_group_check=True)
        osb = pwork.tile([1, 128], F32, tag="osb")
        nc.scalar.activation(osb[:], pg0[0][0:1, 0:128], AF.Sigmoid, bias=fcb[:, 0:1])
        nc.sync.dma_start(d_out.ap()[:], osb[:])

    nc.compile()
    return nc


def _get_program(t_steps: int):
    if t_steps not in _BUILT:
        _BUILT[t_steps] = _build_program(t_steps)
    return _BUILT[t_steps]


def _prep_core_inputs(x, dmask, weights, t_steps):
    """Host-side layout prep for one core's shard. x: [BL, T, I], dmask: [BL, T]."""
    TB = t_steps * BL
    xs = np.ascontiguousarray(
        np.asarray(x, np.float32).transpose(2, 1, 0).reshape(16, TB)
    ).astype(ml_dtypes.bfloat16)
    dbc = np.ascontiguousarray(
        np.broadcast_to(dmask.T.reshape(1, TB), (64, TB))
    ).astype(ml_dtypes.bfloat16)
    return dict(xs=xs, dbc=dbc, **weights)


def _host_weights(w_ih0, w_hh0, b_ih0, b_hh0,
                  w_ih1, w_hh1, b_ih1, b_hh1, fc_w, fc_b):
    b0 = np.asarray(b_ih0, np.float32) + np.asarray(b_hh0, np.float32)
    b1 = np.asarray(b_ih1, np.float32) + np.asarray(b_hh1, np.float32)
    wih0, whh0 = np.asarray(w_ih0, np.float32), np.asarray(w_hh0, np.float32)
    wih1, whh1 = np.asarray(w_ih1, np.float32), np.asarray(w_hh1, np.float32)

    def og_w(wm):  # [4H, K] -> [2g; o] stacked [128, K] (PyTorch i,f,g,o rows)
        return np.concatenate([2.0 * wm[2 * H:3 * H], wm[3 * H:4 * H]], axis=0)

    def og_b(bv):
        return np.concatenate([2.0 * bv[2 * H:3 * H], bv[3 * H:4 * H]])

    def fused0(wx, wh, bv):  # [W_hh; b; W_ih] lhsT [81, 128]
        out = np.empty((81, 128), np.float32)
        out[0:64] = wh.T
        out[64] = bv
        out[65:81] = wx.T
        return out

    def rider1(wx, bv):  # [W_ih1; b1] lhsT [65, 128]
        out = np.empty((65, 128), np.float32)
        out[0:64] = wx.T
        out[64] = bv
        return out

    bf = ml_dtypes.bfloat16
    weights = dict(
        wif0=fused0(wih0[0:2 * H], whh0[0:2 * H], b0[0:2 * H]).astype(bf),
        wog0=fused0(og_w(wih0), og_w(whh0), og_b(b0)).astype(bf),
        w0if1=rider1(wih1[0:2 * H], b1[0:2 * H]).astype(bf),
        w0og1=rider1(og_w(wih1), og_b(b1)).astype(bf),
        whif1=np.ascontiguousarray(whh1[0:2 * H].T).astype(bf),
        whog1=np.ascontiguousarray(og_w(whh1).T).astype(bf),
        fct=np.ascontiguousarray(
            np.asarray(fc_w, np.float32).reshape(1, H).T).astype(bf),
        fcb=np.asarray(fc_b, np.float32).reshape(1, 1),
    )
    return weights


def _run(x, mask, w_ih0, w_hh0, b_ih0, b_hh0,
         w_ih1, w_hh1, b_ih1, b_hh1, fc_w, fc_b, trace=False):
    t_steps = x.shape[1]
    x = np.asarray(x, np.float32)
    mask = np.asarray(mask)

    # d[b, t] = mask[b, t] - mask[b, t+1]  (one-hot at t = len_b - 1)
    m = mask.astype(np.float32)
    d = m - np.concatenate([m[:, 1:], np.zeros((m.shape[0], 1), np.float32)], axis=1)

    weights = _host_weights(w_ih0, w_hh0, b_ih0, b_hh0,
                            w_ih1, w_hh1, b_ih1, b_hh1, fc_w, fc_b)

    nc = _get_program(t_steps)
    in_maps = []
    for c in range(NCORES):
        sl = slice(c * BL, (c + 1) * BL)
        in_maps.append(_prep_core_inputs(x[sl], d[sl], weights, t_steps))

    res = bass_utils.run_bass_kernel_spmd(nc, in_maps, core_ids=list(range(NCORES)),
                                          trace=trace)
    out = np.concatenate([res.results[c]["out"].reshape(BL) for c in range(NCORES)])
    return out.astype(np.float32), res


def kernel(**inputs):
    return _run(**inputs)[0]


def kernel_traced(**inputs):
    return _run(**inputs, trace=True)


# revision 12
# speedup vs baseline: 1.0955x; 1.0898x over previous
"""Trainium2 Bass kernel for a 2-layer masked LSTM + FC + sigmoid head.

Problem shapes (hardcoded): B=1024, T=512, I=16, H=64.
Sharding: pure data parallel, batch 1024 -> 8 cores x 128.

Per-core design (v4 — bf16 matmul path, single sigmoid, Pool fc)
----------------------------------------------------------------
All matmul operands are bf16 (1 PE cycle/row at any moving size, vs
fp32r's >=256 requirement), so the zero-half trick is gone: per step
each layer runs two F=128 matmuls into disjoint PSUM column ranges
[IF | OG] of a [128, 512] bank (2 steps per bank, 2 banks per layer).

Layer 0 needs no separate input projection: gates = [W_hh0; b0; W_ih0]
@ [h0(t-1); 1; x(t)] as a K=81 matmul per gate-block. The rhs is an
8-slot ring [81, 1024] bf16 (slot = 128 cols; h0 rows 0:64, ones row
64, x rows 65:81). Layer 1 lags by 4 steps; its input projection
W_ih1 @ h0 + b1 is batched per step-pair into the IF1/OG1 columns
(start=True), the K=64 recurrent matmuls accumulate on top.

One sigmoid ACT [128, 256] per layer per step covers i, f, 2g, o
(tanh(g) = 2*sigmoid(2g) - 1 with the 2x folded into the g weights);
ig2 needs both halves anyway so splitting only serializes the ACT
queue. Cell update per layer: fc = f*c on GpSimd (parallel with the
Vector engine's ig2 = (sig2g-.5)*i), then c' = 2*ig2 + fc, tanh, and
the h-write (f32 -> bf16 ring cast) on Vector. Emission order per
step keeps h0w ahead of layer-1's trio in the Vector queue.

Masked final-state capture: run unmasked; since d[b,t] = mask[b,t] -
mask[b,t+1] is one-hot over t, capture is a single copy_predicated
per 4-step block (last-write-wins, no accumulate): h2cap[64, 512]
gets h1(t) at each sequence's own length.
"""

from contextlib import ExitStack

import numpy as np
import ml_dtypes

import concourse.bass as bass
import concourse.tile as tile
from concourse import bacc, mybir
from concourse import bass_utils

F32 = mybir.dt.float32
BF16 = mybir.dt.bfloat16
F32R = mybir.dt.float32r
AF = mybir.ActivationFunctionType
OP = mybir.AluOpType

B, T, I, H = 1024, 512, 16, 64
NCORES = 8
BL = B // NCORES  # 128 batch per core
LAG = 4           # layer-1 step lag

_BUILT = {}


def _build_program(t_steps: int):
    nc = bacc.Bacc(
        "TRN2",
        target_bir_lowering=False,
        debug=False,
        enable_asserts=False,
        num_devices=NCORES,
    )

    TB = t_steps * BL  # 65536
    d_xs = nc.dram_tensor("xs", [16, TB], BF16, kind="ExternalInput")
    d_dbc = nc.dram_tensor("dbc", [64, TB], mybir.dt.uint8, kind="ExternalInput")
    wnames = [
        ("wif0", 81), ("wog0", 81),      # [W_hh0; b0; W_ih0] fused lhsT
        ("w0if1", 65), ("w0og1", 65),    # [W_ih1; b1] rider lhsT
        ("whif1", 64), ("whog1", 64),    # W_hh1 lhsT
    ]
    d_w = {}
    for name, k in wnames:
        d_w[name] = nc.dram_tensor(name, [k, 128], BF16, kind="ExternalInput")
    d_fct = nc.dram_tensor("fct", [64, 1], BF16, kind="ExternalInput")
    d_fcb = nc.dram_tensor("fcb", [1, 1], F32, kind="ExternalInput")
    d_out = nc.dram_tensor("out", [1, 128], F32, kind="ExternalOutput")

    NCH = TB // 2048  # 32 dbc chunks of 16 steps

    with tile.TileContext(nc) as tc, ExitStack() as ctx:
        pconst = ctx.enter_context(tc.tile_pool(name="const", bufs=1))
        pstate = ctx.enter_context(tc.tile_pool(name="state", bufs=1))
        ppsum = ctx.enter_context(tc.tile_pool(name="psum", bufs=1, space="PSUM"))
        pwork = ctx.enter_context(tc.tile_pool(name="work", bufs=3))

        # ---- weights ----
        w = {}
        for name, k in wnames:
            w[name] = pconst.tile([k, 128], BF16, tag=name, name=name)
            nc.sync.dma_start(w[name][:], d_w[name].ap()[:])
        fct = pconst.tile([64, 1], BF16, tag="fct")
        nc.sync.dma_start(fct[:], d_fct.ap()[:])
        fcb = pconst.tile([1, 1], F32, tag="fcb")
        nc.sync.dma_start(fcb[:], d_fcb.ap()[:])

        # ---- rings: slot s = 128 cols ----
        # ring0: rows 0:64 h0(s), row 64 ones, rows 65:81 x(s+1)
        ring0 = pstate.tile([81, 8 * 128], BF16, tag="ring0")
        nc.vector.memset(ring0[:], 0.0)
        nc.vector.memset(ring0[64:65, :], 1.0)
        # ring1: rows 0:64 h1(s)
        ring1 = pstate.tile([64, 8 * 128], BF16, tag="ring1")
        nc.vector.memset(ring1[:], 0.0)

        dcb = [pconst.tile([64, 2048], mybir.dt.uint8, tag=f"dcb{i}", name=f"dcb{i}")
               for i in range(3)]
        for j in range(2):
            nc.sync.dma_start(dcb[j][:], d_dbc.ap()[:, j * 2048:(j + 1) * 2048])

        def dma_x(t0, n):
            # x(t0..t0+n-1) -> ring0 slots (t0-1)%8 .. contiguous, rows 65:81
            s0 = (t0 - 1) % 8
            dst = ring0[65:81, 128 * s0:128 * (s0 + n)]
            nc.sync.dma_start(dst, d_xs.ap()[:, t0 * BL:(t0 + n) * BL])

        dma_x(0, 1)   # x(0) -> slot 7
        dma_x(1, 4)   # slots 0..3
        dma_x(5, 4)   # slots 4..7

        c_sb = pstate.tile([128, 256], BF16, tag="csb")
        nc.vector.memset(c_sb[:], 0.0)
        # sig1's bias operand is rewritten (to 0) after each step's c'0:
        # this delays sig1 past c'0 in BOTH the tile scheduler's cost model
        # and on hardware, so tanh0 wins the ACT queue and h0w wins the
        # Vector queue ahead of layer-1's trio (which otherwise wedges in
        # front of them and stretches the critical chain).
        zconst = pconst.tile([128, 1], BF16, tag="zconst")
        nc.vector.memset(zconst[:], 0.0)
        biasg = pstate.tile([128, 1], BF16, tag="biasg")
        nc.vector.memset(biasg[:], 0.0)
        h2cap = pstate.tile([64, 512], BF16, tag="h2cap")
        nc.vector.memset(h2cap[:], 0.0)

        # ---- PSUM: per-layer step-pair banks ----
        pg0 = [ppsum.tile([128, 512], F32, tag=f"pg0{i}", name=f"pg0{i}")
               for i in range(2)]
        pg1 = [ppsum.tile([128, 512], F32, tag=f"pg1{i}", name=f"pg1{i}")
               for i in range(2)]

        mm = nc.tensor.matmul

        for k in range(t_steps + LAG):
            t = k
            tau = k - LAG
            l0 = t < t_steps
            l1 = 0 <= tau < t_steps
            bk0 = pg0[(k // 2) % 2]
            s0c = 256 * (k % 2)
            bk1 = pg1[((k - LAG) // 2) % 2] if k >= LAG else None
            s1c = 256 * ((k - LAG) % 2) if k >= LAG else 0

            # ---- layer-0 matmuls (chain leader) ----
            if l0:
                sp = (t - 1) % 8
                rhs = ring0[0:81, 128 * sp:128 * (sp + 1)]
                mm(bk0[:, s0c:s0c + 128], w["wif0"][:], rhs, start=True,
                   stop=True, skip_group_check=True)
                mm(bk0[:, s0c + 128:s0c + 256], w["wog0"][:], rhs, start=True,
                   stop=True, skip_group_check=True)

            # ---- layer-1 recurrent matmuls ----
            if l1:
                sp = (tau - 1) % 8
                rhs = ring1[0:64, 128 * sp:128 * (sp + 1)]
                mm(bk1[:, s1c:s1c + 128], w["whif1"][:], rhs, start=False,
                   stop=True, skip_group_check=True)
                mm(bk1[:, s1c + 128:s1c + 256], w["whog1"][:], rhs, start=False,
                   stop=True, skip_group_check=True)

            # ---- sigmoids (one per layer) ----
            g_sb = pwork.tile([128, 512], BF16, tag="gsb")
            tc_sb = pwork.tile([128, 256], BF16, tag="tcsb")
            fco = pwork.tile([64, 256], BF16, tag="fco")
            ig2 = pwork.tile([64, 256], BF16, tag="ig2")
            if l0:
                nc.scalar.activation(g_sb[:, 0:256], bk0[:, s0c:s0c + 256],
                                     AF.Sigmoid)


            # cell state is c~ = c/2 (the 2x is folded into tanh's scale),
            # so c' is a plain TT add: c~ = f*c~ + (sig2g - .5)*i
            def cell(ell, ring, st):
                b0c = 256 * ell
                co = 128 * ell
                cin = c_sb[64:128, co:co + 128]
                nc.vector.tensor_tensor(fco[:, co:co + 128],
                                        g_sb[64:128, b0c:b0c + 128], cin,
                                        OP.mult)
                nc.vector.scalar_tensor_tensor(ig2[:, co:co + 128],
                                               g_sb[0:64, b0c + 128:b0c + 256], 0.5,
                                               g_sb[0:64, b0c:b0c + 128],
                                               OP.subtract, OP.mult)
                nc.vector.tensor_tensor(cin, ig2[:, co:co + 128],
                                        fco[:, co:co + 128], OP.add)
                if ell == 0 and l1:
                    # rewrite sig1's bias (to 0.0) with a read of c'0's
                    # output column: the next step's sig1 now trails c'0
                    nc.vector.scalar_tensor_tensor(biasg[:], c_sb[0:128, 0:1],
                                                   0.0, zconst[:],
                                                   OP.mult, OP.add)
                nc.scalar.activation(tc_sb[64:128, co:co + 128], cin, AF.Tanh,
                                     scale=2.0)
                dst = ring[0:64, 128 * st:128 * (st + 1)]
                nc.vector.tensor_tensor(dst, g_sb[64:128, b0c + 128:b0c + 256],
                                        tc_sb[64:128, co:co + 128], OP.mult)

            if l0:
                cell(0, ring0, t % 8)
            if l1:
                # emitted after cell(0) so the bias operand binds to THIS
                # step's biasw (written just after c'0)
                nc.scalar.activation(g_sb[:, 256:512], bk1[:, s1c:s1c + 256],
                                     AF.Sigmoid,
                                     bias=biasg[:] if l0 else 0.0)
                cell(1, ring1, tau % 8)

            # ---- capture: one-hot mask -> predicated copy, per 4 steps ----
            if k % 4 == 3 and k >= LAG + 3:
                c = (k - 3 - LAG) // 4
                t0 = 4 * c
                ch = dcb[(t0 // 16) % 3]
                dsl = ch[:, (t0 % 16) * 128:(t0 % 16) * 128 + 512]
                sp = t0 % 8
                h1s = ring1[0:64, 128 * sp:128 * (sp + 4)]
                nc.vector.copy_predicated(h2cap[:, 0:256], dsl[:, 0:256],
                                          h1s[:, 0:256])
                nc.vector.copy_predicated(h2cap[:, 256:512], dsl[:, 256:512],
                                          h1s[:, 256:512])

            # ---- layer-1 input projection, 3 steps ahead of use ----
            if k % 2 == 1:
                q = (k - 1) // 2
                if 0 <= q < t_steps // 2:
                    tb = pg1[q % 2]
                    rr = tb[:].rearrange("p (s c) -> p s c", c=256)
                    t0 = 2 * q
                    sp = t0 % 8
                    rhs = ring0[0:65, 128 * sp:128 * (sp + 2)]
                    mm(rr[:, :, 0:128], w["w0if1"][:], rhs, start=True,
                       stop=False, skip_group_check=True)
                    mm(rr[:, :, 128:256], w["w0og1"][:], rhs, start=True,
                       stop=False, skip_group_check=True)

            # ---- input streaming ----
            if k % 4 == 0 and k >= 8 and k + 1 < t_steps:
                n = min(4, t_steps - (k + 1))
                dma_x(k + 1, n)
            if k % 16 == 4 and k // 16 + 2 < NCH:
                j = k // 16 + 2
                nc.sync.dma_start(dcb[j % 3][:], d_dbc.ap()[:, j * 2048:(j + 1) * 2048])

        # ---------- FC + sigmoid head ----------
        hfold = pwork.tile([64, 256], BF16, tag="hfold")
        nc.vector.tensor_tensor(hfold[:], h2cap[:, 0:256], h2cap[:, 256:512], OP.add)
        h2 = pwork.tile([64, 128], BF16, tag="h2")
        nc.vector.tensor_tensor(h2[:], hfold[:, 0:128], hfold[:, 128:256], OP.add)
        mm(pg0[0][0:1, 0:128], fct[:], h2[:], start=True, stop=True,
           skip_group_check=True)
        osb = pwork.tile([1, 128], F32, tag="osb")
        nc.scalar.activation(osb[:], pg0[0][0:1, 0:128], AF.Sigmoid, bias=fcb[:, 0:1])
        nc.sync.dma_start(d_out.ap()[:], osb[:])

    nc.compile()
    return nc


def _get_program(t_steps: int):
    if t_steps not in _BUILT:
        _BUILT[t_steps] = _build_program(t_steps)
    return _BUILT[t_steps]


def _prep_core_inputs(x, dmask, weights, t_steps):
    """Host-side layout prep for one core's shard. x: [BL, T, I], dmask: [BL, T]."""
    TB = t_steps * BL
    xs = np.ascontiguousarray(
        np.asarray(x, np.float32).transpose(2, 1, 0).reshape(16, TB)
    ).astype(ml_dtypes.bfloat16)
    dbc = np.ascontiguousarray(
        np.broadcast_to(dmask.T.reshape(1, TB), (64, TB))
    ).astype(np.uint8)
    return dict(xs=xs, dbc=dbc, **weights)


def _host_weights(w_ih0, w_hh0, b_ih0, b_hh0,
                  w_ih1, w_hh1, b_ih1, b_hh1, fc_w, fc_b):
    b0 = np.asarray(b_ih0, np.float32) + np.asarray(b_hh0, np.float32)
    b1 = np.asarray(b_ih1, np.float32) + np.asarray(b_hh1, np.float32)
    wih0, whh0 = np.asarray(w_ih0, np.float32), np.asarray(w_hh0, np.float32)
    wih1, whh1 = np.asarray(w_ih1, np.float32), np.asarray(w_hh1, np.float32)

    def og_w(wm):  # [4H, K] -> [2g; o] stacked [128, K] (PyTorch i,f,g,o rows)
        return np.concatenate([2.0 * wm[2 * H:3 * H], wm[3 * H:4 * H]], axis=0)

    def og_b(bv):
        return np.concatenate([2.0 * bv[2 * H:3 * H], bv[3 * H:4 * H]])

    def fused0(wx, wh, bv):  # [W_hh; b; W_ih] lhsT [81, 128]
        out = np.empty((81, 128), np.float32)
        out[0:64] = wh.T
        out[64] = bv
        out[65:81] = wx.T
        return out

    def rider1(wx, bv):  # [W_ih1; b1] lhsT [65, 128]
        out = np.empty((65, 128), np.float32)
        out[0:64] = wx.T
        out[64] = bv
        return out

    bf = ml_dtypes.bfloat16
    weights = dict(
        wif0=fused0(wih0[0:2 * H], whh0[0:2 * H], b0[0:2 * H]).astype(bf),
        wog0=fused0(og_w(wih0), og_w(whh0), og_b(b0)).astype(bf),
        w0if1=rider1(wih1[0:2 * H], b1[0:2 * H]).astype(bf),
        w0og1=rider1(og_w(wih1), og_b(b1)).astype(bf),
        whif1=np.ascontiguousarray(whh1[0:2 * H].T).astype(bf),
        whog1=np.ascontiguousarray(og_w(whh1).T).astype(bf),
        fct=np.ascontiguousarray(
            np.asarray(fc_w, np.float32).reshape(1, H).T).astype(bf),
        fcb=np.asarray(fc_b, np.float32).reshape(1, 1),
    )
    return weights


def _run(x, mask, w_ih0, w_hh0, b_ih0, b_hh0,
         w_ih1, w_hh1, b_ih1, b_hh1, fc_w, fc_b, trace=False):
    t_steps = x.shape[1]
    x = np.asarray(x, np.float32)
    mask = np.asarray(mask)

    # d[b, t] = mask[b, t] - mask[b, t+1]  (one-hot at t = len_b - 1)
    m = mask.astype(np.float32)
    d = m - np.concatenate([m[:, 1:], np.zeros((m.shape[0], 1), np.float32)], axis=1)

    weights = _host_weights(w_ih0, w_hh0, b_ih0, b_hh0,
                            w_ih1, w_hh1, b_ih1, b_hh1, fc_w, fc_b)

    nc = _get_program(t_steps)
    in_maps = []
    for c in range(NCORES):
        sl = slice(c * BL, (c + 1) * BL)
        in_maps.append(_prep_core_inputs(x[sl], d[sl], weights, t_steps))

    res = bass_utils.run_bass_kernel_spmd(nc, in_maps, core_ids=list(range(NCORES)),
                                          trace=trace)
    out = np.concatenate([res.results[c]["out"].reshape(BL) for c in range(NCORES)])
    return out.astype(np.float32), res


def kernel(**inputs):
    return _run(**inputs)[0]


def kernel_traced(**inputs):
    return _run(**inputs, trace=True)
